# revision 14
# baseline (speedup 1.0000x reference)
"""Banded local-linear layer (nn_LocalLinearLayer) on 8 trn2 NeuronCores.

out[b, o, c] = sum_p W[o, p] * xpad[b, c, p] + bias[o],  band p in [o, o+25)
xpad = edge-replicate pad of x along L (first/last 12 rows duplicated).

Strategy (v9):
  - Tensor-parallel over L: 40 global output tiles of 104 rows (K=128 window);
    core s owns tiles [5s, 5s+5) and only its slice of the banded weight.
  - The per-tile weight block [128, 104] is PACKED at the head of the x tile
    ([104 w | 2048 x] = 4304 B lines), so each tile arrives in one large-line
    DMA; tile 0 is split in two so the first matmul can start early.
  - Per tile: 4 matmuls (N=512) into single-bank PSUM tiles (bufs=8 so PSUM
    recycle latency never caps the matmul rate), each drained by a pure copy
    (vector/scalar alternating) into fp16 out tiles. Bias is added on the
    HOST during gather (it only depends on the output row).
  - Rings: sync = x0a,x0b,x2,x4 then out1,out3,out4a; scalar = x1,x3,out4b;
    gpsimd (SW) = out0,out2.
  - fp16 operands and output, fp32 PSUM.
"""

import sys

for _p in ("/opt/trn_rl_repo",):
    if _p not in sys.path:
        sys.path.insert(0, _p)

import numpy as np

import concourse.bass as bass
import concourse.tile as tile
from concourse import bacc, mybir
from concourse.bass_utils import run_bass_kernel_spmd

L = 4096
WIN = 25
PAD = (WIN - 1) // 2  # 12
PADDED = L + 2 * PAD  # 4120
B = 32
C = 64
NCORES = 8
P = 128
M = P - (WIN - 1)  # 104 output rows per tile
NT = (L + M - 1) // M  # 40 global tiles
TPC = NT // NCORES  # 5 tiles per core
N = B * C  # 2048 free dim
NW = M + N  # 2152: packed weight columns + x tile
CH = 512  # matmul moving free size (1 bank)
SPLIT0 = M + N // 2  # 1128: first DMA of tile 0 covers w + x[:1024]

F32 = mybir.dt.float32
F16 = mybir.dt.float16


def _host_weights(W: np.ndarray):
    o = np.arange(L)[:, None]
    p = np.arange(PADDED)[None, :]
    Wm = np.where((p >= o) & (p < o + WIN), W, 0.0).astype(np.float32)
    # wb[k, t, m] = Wm[t*104+m, t*104+k], zero-padded out of range
    wb = np.zeros((P, NT, M), np.float32)
    for t in range(NT):
        mt = min(M, L - t * M)
        kt = min(P, PADDED - t * M)
        wb[:kt, t, :mt] = Wm[t * M : t * M + mt, t * M : t * M + kt].T
    return wb.astype(np.float16)


def _host_x(x: np.ndarray):
    """x [B, L, C] f32 -> [P, NT, B, C] f16 in xpad-tile layout."""
    xp = np.concatenate([x[:, :PAD], x, x[:, -PAD:]], axis=1).astype(np.float16)
    xh = np.zeros((P, NT, B, C), np.float16)
    for t in range(NT):
        kt = min(P, PADDED - t * M)
        xh[:kt, t] = xp[:, t * M : t * M + kt].transpose(1, 0, 2)
    return xh


def _build_nc():
    nc = bacc.Bacc("TRN2", target_bir_lowering=False, debug=False, num_devices=NCORES)
    xwb_d = nc.dram_tensor("xwb", [P, TPC, NW], F16, kind="ExternalInput").ap()
    out_d = nc.dram_tensor("out", [M, TPC, N], F16, kind="ExternalOutput").ap()

    with tile.TileContext(nc) as tc:
        with (
            tc.tile_pool(name="main", bufs=1) as pool,
            tc.tile_pool(name="ps", bufs=7, space=bass.MemorySpace.PSUM) as pspool,
            tc.tile_pool(name="wps", bufs=1, space=bass.MemorySpace.PSUM) as wpool,
        ):
            xs = [pool.tile([P, NW], F16, name=f"xs{j}") for j in range(TPC)]
            outs = [pool.tile([M, N], F16, name=f"outs{j}") for j in range(TPC)]
            warm = pool.tile([P, CH], F16, name="warm")

            for j in range(TPC):
                ring = nc.sync if j % 2 == 0 else nc.scalar
                ring.dma_start(xs[j][:], xwb_d[:, j])

            # p-state warmup: keep the PE busy before the first x tile lands so
            # real matmuls start at full clock (ramp needs ~3us of activity)
            nc.gpsimd.memset(warm[:], 0.0)
            for _ in range(5):
                wps = wpool.tile([P, CH], F32)
                nc.tensor.matmul(
                    wps[:], warm[:, :P], warm[:], start=True, stop=True
                )

            out_rings = [nc.gpsimd, nc.scalar, nc.gpsimd, nc.sync]
            di = 0
            for j in range(TPC):
                for c in range(4):
                    ps = pspool.tile([M, CH], F32)
                    nc.tensor.matmul(
                        ps[:],
                        xs[j][:, :M],
                        xs[j][:, M + c * CH : M + (c + 1) * CH],
                        start=True,
                        stop=True,
                    )
                    if di % 2 == 0:
                        nc.vector.tensor_scalar_add(
                            outs[j][:, c * CH : (c + 1) * CH], ps[:], 0.0
                        )
                    else:
                        nc.scalar.copy(outs[j][:, c * CH : (c + 1) * CH], ps[:])
                    di += 1
                if j < TPC - 1:
                    out_rings[j].dma_start(out_d[:, j], outs[j][:])
                else:
                    # last tile: split BY PARTITION across both HW rings (keeps
                    # 4KB lines) to shorten the tail
                    nc.sync.dma_start(out_d[: M // 2, j], outs[j][: M // 2])
                    nc.scalar.dma_start(out_d[M // 2 :, j], outs[j][M // 2 :])

    nc.compile()
    return nc


_NC = None


def _get_nc():
    global _NC
    if _NC is None:
        _NC = _build_nc()
    return _NC


def _make_in_maps(x, W, b=None):
    wb = _host_weights(np.asarray(W, dtype=np.float32))  # [P, NT, M] f16
    xh = _host_x(np.asarray(x, dtype=np.float32))  # [P, NT, B, C] f16
    maps = []
    for s in range(NCORES):
        xwb = np.empty((P, TPC, NW), np.float16)
        xwb[:, :, :M] = wb[:, TPC * s : TPC * (s + 1)]
        xwb[:, :, M:] = xh[:, TPC * s : TPC * (s + 1)].reshape(P, TPC, N)
        maps.append({"xwb": xwb})
    return maps


def _gather(results, b):
    oh = np.concatenate(
        [r["out"].reshape(M, TPC, B, C) for r in results], axis=1
    )  # [104, 40, B, C]
    out = np.empty((B, L, C), np.float32)
    for t in range(NT):
        mt = min(M, L - t * M)
        out[:, t * M : t * M + mt] = oh[:mt, t].transpose(1, 0, 2)
    out += np.asarray(b, dtype=np.float32)[None, :, None]
    return out


def kernel(x: np.ndarray, W: np.ndarray, b: np.ndarray) -> np.ndarray:
    nc = _get_nc()
    res = run_bass_kernel_spmd(nc, _make_in_maps(x, W), list(range(NCORES)))
    return _gather(res.results, b)


if __name__ == "__main__":
    rng = np.random.default_rng(0)
    x = rng.standard_normal((B, L, C), dtype=np.float32)
    W = rng.standard_normal((L, PADDED), dtype=np.float32) * 0.02
    b = rng.standard_normal((L,), dtype=np.float32) * 0.02
    print(kernel(x, W, b).shape)
